# revision 1
# baseline (speedup 1.0000x reference)
"""FFTBlock (attention + conv-FFN transformer block) on 8 Trainium2 NeuronCores.

Data-parallel over batch: 16 batch items -> 2 per core. Each core runs the
full block (MHA + LN + conv1d-FFN + LN) on its 2 batch items.

Schedule (per core): a single woven PE stream designed so the tensor engine
never idles long enough for the HAM clock gate to re-throttle:

  P0: qkv(b0)                               (weights host-packed partition-major
                                             so every weight DMA is rectangular)
  P1: attn-scores/AV(b0)  ~weave~ qkv(b1)
      outproj(b0)         ~weave~ conv1(b0, qc=0)   <- dense PE filler
  P2: attn-scores/AV(b1)  ~weave~ conv1(b0, qc=1)
      outproj(b1)         ~weave~ conv1(b0) leftovers
  P3: conv1(b1), conv2(b0), conv2(b1)       (w2 prefetched at P3 start)

Other structural points:
  - residual (xn + bo) precomputed into SBUF off the critical path; LN chains
    use bn_stats/bn_aggr + Rsqrt; gamma/beta applied only if nontrivial.
  - scores matmuls zero-pad the DH=192 contraction to 2x128 partitions
    (64-partition matmuls measure ~1.5x slower than 128).
  - x1n (post-LN1) stored bf16 so PE transposes for the conv input run at
    1 cycle/row; softmax denominator via ones-column in V.
  - PSUM statically planned at exactly 8 banks.
"""

import sys

sys.path.insert(0, "/opt/trn_rl_repo")

import math
from contextlib import ExitStack

import ml_dtypes
import numpy as np

import concourse.bass as bass
import concourse.mybir as mybir
import concourse.tile as tile
from concourse import bacc
from concourse.bass_utils import run_bass_kernel_spmd
from concourse.masks import make_identity

BF16 = mybir.dt.bfloat16
F32 = mybir.dt.float32
AF = mybir.ActivationFunctionType
ALU = mybir.AluOpType

B, S, D, H, DH, F, K = 16, 1024, 384, 2, 192, 1536, 9
NCORES = 8
NB = B // NCORES  # batch items per core
EPS = 1e-5
ISCALE = 1.0 / math.sqrt(D)  # NOTE: reference scales by sqrt(d_model)
SP = S + 8  # padded sequence length (4 left, 4 right)
DC = D // 128  # 3 d-chunks
FT = F // 128  # 12 filter tiles
ST = S // 128  # 8 seq tiles of 128
SQ = S // 512  # 2 seq chunks of 512

_CACHE = {}


def _bcast(ap, p=128):
    return bass.AP(tensor=ap.tensor, offset=ap.offset, ap=[[0, p]] + list(ap.ap))


def weave(a, b, frac=1.0):
    # proportional merge of two unit lists; emits every closure.
    # `frac`: a is fully emitted once frac of b has been emitted, so the tail
    # of b covers a's trailing cross-engine latency with dense PE work.
    nb = max(1, int(len(b) * frac))
    ia = ib = 0
    while ia < len(a) or ib < len(b):
        if ib >= len(b) or (ia < len(a) and ia * (nb + 1) <= ib * (len(a) + 1)):
            a[ia]()
            ia += 1
        else:
            b[ib]()
            ib += 1


def _emit(nc, trivial_g1, trivial_g2):
    # ---- DRAM I/O (all weights host-packed partition-major) ----
    d = {}
    d["xT_d"] = nc.dram_tensor("xT", [NB, DC, 128, S], BF16, kind="ExternalInput")
    d["xn_d"] = nc.dram_tensor("xn", [NB, ST, 128, D], BF16, kind="ExternalInput")
    d["mT_d"] = nc.dram_tensor("mT", [NB, ST, 128, S], BF16, kind="ExternalInput")
    d["wq_d"] = nc.dram_tensor("wq", [128, H, DC, DH], BF16, kind="ExternalInput")
    d["wk_d"] = nc.dram_tensor("wk", [128, H, DC, DH], BF16, kind="ExternalInput")
    d["wv_d"] = nc.dram_tensor("wv", [128, H, DC, DH], BF16, kind="ExternalInput")
    d["wo_d"] = nc.dram_tensor("wo", [128, 4, D], BF16, kind="ExternalInput")
    d["wc1_d"] = nc.dram_tensor("wc1", [128, K, DC, F], BF16, kind="ExternalInput")
    d["wc2_d"] = nc.dram_tensor("wc2", [128, K, FT, D], BF16, kind="ExternalInput")
    d["bqk_d"] = nc.dram_tensor("bqk", [128, 2, H, 2], F32, kind="ExternalInput")
    d["bv_d"] = nc.dram_tensor("bv", [H, DH], F32, kind="ExternalInput")
    d["bo_d"] = nc.dram_tensor("bo", [D], F32, kind="ExternalInput")
    d["bc1_d"] = nc.dram_tensor("bc1t", [128, FT], F32, kind="ExternalInput")
    d["bc2_d"] = nc.dram_tensor("bc2", [D], F32, kind="ExternalInput")
    d["g1_d"] = nc.dram_tensor("g1", [D], F32, kind="ExternalInput")
    d["be1_d"] = nc.dram_tensor("be1", [D], F32, kind="ExternalInput")
    d["g2_d"] = nc.dram_tensor("g2", [D], F32, kind="ExternalInput")
    d["be2_d"] = nc.dram_tensor("be2", [D], F32, kind="ExternalInput")
    d["y_d"] = nc.dram_tensor("y", [NB, ST, 128, D], F32, kind="ExternalOutput")
    d["trivial_g1"] = trivial_g1
    d["trivial_g2"] = trivial_g2

    with tile.TileContext(nc) as tc:
        _body(nc, tc, d)
    nc.finalize()
    return nc


def _body(nc, tc, d):
    xT_d, xn_d, mT_d = d["xT_d"], d["xn_d"], d["mT_d"]
    wq_d, wk_d, wv_d, wo_d = d["wq_d"], d["wk_d"], d["wv_d"], d["wo_d"]
    wc1_d, wc2_d = d["wc1_d"], d["wc2_d"]
    bqk_d, bv_d, bo_d, bc1_d, bc2_d = (
        d["bqk_d"], d["bv_d"], d["bo_d"], d["bc1_d"], d["bc2_d"],
    )
    g1_d, be1_d, g2_d, be2_d, y_d = d["g1_d"], d["be1_d"], d["g2_d"], d["be2_d"], d["y_d"]
    triv1, triv2 = d["trivial_g1"], d["trivial_g2"]

    with ExitStack() as ctx:
        const = ctx.enter_context(tc.tile_pool(name="const", bufs=1))
        persist = ctx.enter_context(tc.tile_pool(name="persist", bufs=1))

        # ---- long-lived conv pools first (LIFO stack: created before actx) ----
        w1p = ctx.enter_context(tc.tile_pool(name="w1p", bufs=2))
        psF = ctx.enter_context(tc.tile_pool(name="psF", bufs=2, space="PSUM"))
        hT0p = ctx.enter_context(tc.tile_pool(name="hT0p", bufs=1, side="right"))

        # ---- attention-lifetime pools (closed before P3 to fit w2) ----
        actx = ctx.enter_context(ExitStack())

        # ---- critical-path DMAs first: what qkv(b0) needs ----
        xtp = actx.enter_context(tc.tile_pool(name="xtp", bufs=1))
        xT_sb = xtp.tile([128, NB, DC, S], BF16, tag="xT")
        nc.sync.dma_start(xT_sb[:, 0], xT_d[0].rearrange("c p s -> p c s"))
        wq_sb = const.tile([128, H, DC, DH], BF16, tag="wq")
        nc.sync.dma_start(wq_sb[:], wq_d[:])
        wk_sb = const.tile([128, H, DC, DH], BF16, tag="wk")
        nc.sync.dma_start(wk_sb[:], wk_d[:])
        bqk_sb = const.tile([128, 2, H, 2], F32, tag="bqk")
        nc.sync.dma_start(bqk_sb[:], bqk_d[:])
        wv_sb = const.tile([128, H, DC, DH], BF16, tag="wv")
        nc.sync.dma_start(wv_sb[:], wv_d[:])
        bv_sb = const.tile([128, H, DH], F32, tag="bv")
        nc.sync.dma_start(bv_sb[:], _bcast(bv_d[:]))
        nc.sync.dma_start(xT_sb[:, 1], xT_d[1].rearrange("c p s -> p c s"))

        # residual (xn + bo), prefolded off the critical path
        xnp = actx.enter_context(tc.tile_pool(name="xnp", bufs=1))
        xnbo = xnp.tile([128, NB, ST, D], BF16, tag="xnbo")
        nc.sync.dma_start(xnbo[:], xn_d[:].rearrange("b s p d -> p b s d"))
        bo_sb = const.tile([128, D], F32, tag="bo")
        nc.sync.dma_start(bo_sb[:], _bcast(bo_d[:]))

        # remaining constants (off the critical path)
        wo_sb = const.tile([128, 4, D], BF16, tag="wo")
        nc.sync.dma_start(wo_sb[:], wo_d[:])
        ident = const.tile([128, 128], BF16, tag="ident")
        make_identity(nc, ident[:])
        bc1_sb = const.tile([128, FT], F32, tag="bc1")
        nc.sync.dma_start(bc1_sb[:], bc1_d[:])
        bc2_sb = const.tile([128, D], F32, tag="bc2")
        nc.sync.dma_start(bc2_sb[:], _bcast(bc2_d[:]))
        if not triv1:
            g1_sb = const.tile([128, D], F32, tag="g1")
            nc.sync.dma_start(g1_sb[:], _bcast(g1_d[:]))
            be1_sb = const.tile([128, D], F32, tag="be1")
            nc.sync.dma_start(be1_sb[:], _bcast(be1_d[:]))
        if not triv2:
            g2_sb = const.tile([128, D], F32, tag="g2")
            nc.sync.dma_start(g2_sb[:], _bcast(g2_d[:]))
            be2_sb = const.tile([128, D], F32, tag="be2")
            nc.sync.dma_start(be2_sb[:], _bcast(be2_d[:]))
        eps_sb = const.tile([128, 1], F32, tag="eps")
        nc.vector.memset(eps_sb[:], EPS)

        x1T = persist.tile([128, NB, DC, SP], BF16, tag="x1T")
        x1n = persist.tile([128, NB, ST, D], BF16, tag="x1n")
        for b in range(NB):
            nc.gpsimd.memset(x1T[:, b, :, 0:4], 0.0)
            nc.gpsimd.memset(x1T[:, b, :, 4 + S : SP], 0.0)

        # ---- global PSUM plan: exactly 8 banks ----
        # psF(2): conv1 + all qkv | psS(1) scores | av0(1)+av1(1)
        # psP(1) outproj | psT(2) transposes
        psS = actx.enter_context(tc.tile_pool(name="psS", bufs=1, space="PSUM"))
        psAV = actx.enter_context(tc.tile_pool(name="psAV", bufs=1, space="PSUM"))
        psP = actx.enter_context(tc.tile_pool(name="psP", bufs=1, space="PSUM"))
        psT = actx.enter_context(tc.tile_pool(name="psT", bufs=2, space="PSUM"))

        QT, KT, VV, ON = {}, {}, {}, {}

        def qkv_units(b, qkp, pspool):
            units = []
            for h in range(H):
                qt = qkp.tile([128, 2, S], BF16, tag=f"qt{b}{h}")
                kt = qkp.tile([128, 2, S], BF16, tag=f"kt{b}{h}")
                vv = qkp.tile([128, ST, DH + 1], BF16, tag=f"vv{b}{h}")
                QT[b, h], KT[b, h], VV[b, h] = qt, kt, vv
                # zero-pad rows 64:128 of the second DH chunk so scores can
                # run full-128-partition contractions
                nc.gpsimd.memset(qt[64:128, 1, :], 0.0)
                nc.gpsimd.memset(kt[64:128, 1, :], 0.0)
                for wsb, bi, dst in ((wq_sb, 0, qt), (wk_sb, 1, kt)):
                    for mc, (m0, msz) in enumerate(((0, 128), (128, 64))):
                        for qc in range(SQ):
                            def u(b=b, h=h, wsb=wsb, bi=bi, dst=dst, m0=m0,
                                  msz=msz, mc=mc, qc=qc):
                                ps = pspool.tile([128, 512], F32, tag="c1")
                                for dc in range(DC):
                                    nc.tensor.matmul(
                                        ps[:msz, :],
                                        lhsT=wsb[:, h, dc, m0 : m0 + msz],
                                        rhs=xT_sb[:, b, dc, qc * 512 : qc * 512 + 512],
                                        start=(dc == 0),
                                        stop=(dc == DC - 1),
                                    )
                                nc.scalar.activation(
                                    out=dst[:msz, mc, qc * 512 : qc * 512 + 512],
                                    in_=ps[:msz, :],
                                    func=AF.Identity,
                                    bias=bqk_sb[:msz, bi, h, mc : mc + 1],
                                    scale=1.0,
                                )
                            units.append(u)
                for st in range(ST):
                    def u(b=b, h=h, vv=vv, st=st):
                        ps = pspool.tile([128, 512], F32, tag="c1")
                        for dc in range(DC):
                            nc.tensor.matmul(
                                ps[:, :DH],
                                lhsT=xT_sb[:, b, dc, st * 128 : st * 128 + 128],
                                rhs=wv_sb[:, h, dc, :],
                                start=(dc == 0),
                                stop=(dc == DC - 1),
                            )
                        nc.vector.tensor_add(
                            out=vv[:, st, 0:DH], in0=ps[:, :DH], in1=bv_sb[:, h, :]
                        )
                        nc.gpsimd.memset(vv[:, st, DH : DH + 1], 1.0)
                    units.append(u)
            return units

        def scoreav_units(b, expp, mskp, smal, attn):
            units = []
            for h in range(H):
                expT = expp.tile([128, ST, S], BF16, tag="expT")
                mtiles = {}
                # prefetch the first masks of this head
                def mhead(b=b, mtiles=mtiles):
                    for kc in range(2):
                        mt = mskp.tile([128, S], BF16, tag="mt")
                        mtiles[kc] = mt
                        nc.sync.dma_start(mt[:], mT_d[b, kc])
                units.append(mhead)
                for kc in range(ST):
                    for qc in range(SQ):
                        def u(b=b, h=h, expT=expT, kc=kc, qc=qc, mtiles=mtiles):
                            qt, kt = QT[b, h], KT[b, h]
                            qs = slice(qc * 512, qc * 512 + 512)
                            ps = psS.tile([128, 512], F32, tag="sc")
                            for mc in range(2):
                                nc.tensor.matmul(
                                    ps[:],
                                    lhsT=kt[:, mc, kc * 128 : kc * 128 + 128],
                                    rhs=qt[:, mc, qs],
                                    start=(mc == 0),
                                    stop=(mc == 1),
                                )
                            nc.scalar.activation(
                                out=expT[:, kc, qs], in_=ps[:], func=AF.Exp,
                                scale=ISCALE,
                            )
                            if qc == 0 and kc + 2 < ST:
                                mt = mskp.tile([128, S], BF16, tag="mt")
                                mtiles[kc + 2] = mt
                                nc.sync.dma_start(mt[:], mT_d[b, kc + 2])
                            nc.vector.tensor_mul(
                                out=expT[:, kc, qs], in0=expT[:, kc, qs],
                                in1=mtiles[kc][:, qs],
                            )
                        units.append(u)
                onrm = attn.tile([128, 2, S], BF16, tag=f"on{b}{h}")
                ON[b, h] = onrm
                for qc in range(SQ):
                    def u(b=b, h=h, expT=expT, onrm=onrm, qc=qc):
                        vv = VV[b, h]
                        qs = slice(qc * 512, qc * 512 + 512)
                        ps0 = psAV.tile([128, 512], F32, tag="av0")
                        ps1 = psAV.tile([65, 512], F32, tag="av1")
                        for kc in range(ST):
                            nc.tensor.matmul(
                                ps0[:],
                                lhsT=vv[:, kc, 0:128],
                                rhs=expT[:, kc, qs],
                                start=(kc == 0),
                                stop=(kc == ST - 1),
                            )
                            nc.tensor.matmul(
                                ps1[:],
                                lhsT=vv[:, kc, 128 : DH + 1],
                                rhs=expT[:, kc, qs],
                                start=(kc == 0),
                                stop=(kc == ST - 1),
                            )
                        rc = smal.tile([1, 512], F32, tag="rc")
                        nc.vector.reciprocal(rc[:], ps1[64:65, :])
                        rb = smal.tile([128, 512], F32, tag="rb")
                        nc.gpsimd.partition_broadcast(rb[:], rc[:])
                        nc.vector.tensor_mul(out=onrm[:, 0, qs], in0=ps0[:], in1=rb[:])
                        nc.vector.tensor_mul(
                            out=onrm[:64, 1, qs], in0=ps1[:64, :], in1=rb[:64, :]
                        )
                    units.append(u)
            return units

        chunks = ((0, 0, 128, 0), (0, 1, 64, 1), (1, 0, 128, 2), (1, 1, 64, 3))

        def outproj_units(b, lnp):
            units = []
            for st in range(ST):
                def u(b=b, st=st):
                    ps = psP.tile([128, D], F32, tag="op")
                    for i, (h, c, ksz, wc) in enumerate(chunks):
                        nc.tensor.matmul(
                            ps[:],
                            lhsT=ON[b, h][:ksz, c, st * 128 : st * 128 + 128],
                            rhs=wo_sb[:ksz, wc, :],
                            start=(i == 0),
                            stop=(i == 3),
                        )
                    t = lnp.tile([128, D], F32, tag="t")
                    nc.vector.tensor_add(out=t[:], in0=ps[:], in1=xnbo[:, b, st, :])
                    stats = lnp.tile([128, 6], F32, tag="st")
                    nc.vector.bn_stats(out=stats[:], in_=t[:])
                    mv = lnp.tile([128, 2], F32, tag="mv")
                    nc.vector.bn_aggr(out=mv[:], in_=stats[:])
                    sd = lnp.tile([128, 1], F32, tag="sd")
                    nc.scalar.activation(
                        out=sd[:], in_=mv[:, 1:2], func=AF.Sqrt, bias=eps_sb[:],
                    )
                    nc.vector.reciprocal(sd[:], sd[:])
                    xv = x1n[:, b, st, :]
                    nc.vector.tensor_scalar(
                        out=xv, in0=t[:], scalar1=mv[:, 0:1], scalar2=sd[:],
                        op0=ALU.subtract, op1=ALU.mult,
                    )
                    if not triv1:
                        nc.gpsimd.tensor_mul(out=xv, in0=xv, in1=g1_sb[:])
                        nc.gpsimd.tensor_add(out=xv, in0=xv, in1=be1_sb[:])
                    for dc in range(DC):
                        tp = psT.tile([128, 128], BF16, tag="tp")
                        nc.tensor.transpose(
                            tp[:], x1n[:, b, st, dc * 128 : dc * 128 + 128], ident[:]
                        )
                        nc.scalar.copy(
                            out=x1T[:, b, dc, 4 + st * 128 : 4 + st * 128 + 128],
                            in_=tp[:],
                        )
                units.append(u)
            return units

        def conv1_units(b, qcs, w1p, psF, hT):
            units = []
            state = {}
            for ft in range(FT):
                def udma(ft=ft, state=state):
                    w1 = w1p.tile([128, K, DC, 128], BF16, tag="w1")
                    state["w1"] = w1
                    nc.sync.dma_start(w1[:], wc1_d[:, :, :, ft * 128 : ft * 128 + 128])
                units.append(udma)
                for qc in qcs:
                    def u(b=b, ft=ft, qc=qc, state=state):
                        w1 = state["w1"]
                        ps = psF.tile([128, 512], F32, tag="c1")
                        idx = 0
                        for k9 in range(K):
                            for dc in range(DC):
                                nc.tensor.matmul(
                                    ps[:],
                                    lhsT=w1[:, k9, dc, :],
                                    rhs=x1T[:, b, dc, qc * 512 + k9 : qc * 512 + k9 + 512],
                                    start=(idx == 0),
                                    stop=(idx == K * DC - 1),
                                )
                                idx += 1
                        nc.scalar.activation(
                            out=hT[:, ft, 4 + qc * 512 : 4 + qc * 512 + 512],
                            in_=ps[:],
                            func=AF.Relu,
                            bias=bc1_sb[:, ft : ft + 1],
                            scale=1.0,
                        )
                    units.append(u)
            return units

        def conv2(b, psG, ln2, hT, w2, x1nc7=None):
            for st in range(ST):
                ps = psG.tile([128, D], F32, tag="c2")
                idx = 0
                for k9 in range(K):
                    for fc in range(FT):
                        nc.tensor.matmul(
                            ps[:],
                            lhsT=hT[:, fc, st * 128 + k9 : st * 128 + k9 + 128],
                            rhs=w2[:, k9, fc, :],
                            start=(idx == 0),
                            stop=(idx == K * FT - 1),
                        )
                        idx += 1
                t = ln2.tile([128, D], F32, tag="t")
                if x1nc7 is not None and st == ST - 1:
                    nc.vector.tensor_add(out=t[:], in0=ps[:], in1=x1nc7[:])
                else:
                    nc.vector.tensor_add(out=t[:], in0=ps[:], in1=x1n[:, b, st, :])
                    nc.vector.tensor_add(out=t[:], in0=t[:], in1=bc2_sb[:])
                stats = ln2.tile([128, 6], F32, tag="st")
                nc.vector.bn_stats(out=stats[:], in_=t[:])
                mv = ln2.tile([128, 2], F32, tag="mv")
                nc.vector.bn_aggr(out=mv[:], in_=stats[:])
                sd = ln2.tile([128, 1], F32, tag="sd")
                nc.scalar.activation(
                    out=sd[:], in_=mv[:, 1:2], func=AF.Sqrt, bias=eps_sb[:],
                )
                nc.vector.reciprocal(sd[:], sd[:])
                ot = ln2.tile([128, D], F32, tag="o")
                nc.vector.tensor_scalar(
                    out=ot[:], in0=t[:], scalar1=mv[:, 0:1], scalar2=sd[:],
                    op0=ALU.subtract, op1=ALU.mult,
                )
                if not triv2:
                    nc.vector.tensor_mul(out=ot[:], in0=ot[:], in1=g2_sb[:])
                    nc.vector.tensor_add(out=ot[:], in0=ot[:], in1=be2_sb[:])
                nc.sync.dma_start(y_d[b, st], ot[:])

        # ================== emission ==================
        qkp0 = actx.enter_context(tc.tile_pool(name="qkp0", bufs=1))
        qkp1 = actx.enter_context(tc.tile_pool(name="qkp1", bufs=1))
        expp = actx.enter_context(tc.tile_pool(name="expp", bufs=1))
        mskp = actx.enter_context(tc.tile_pool(name="mskp", bufs=4))
        smal = actx.enter_context(tc.tile_pool(name="smal", bufs=2))
        lnp = actx.enter_context(tc.tile_pool(name="lnp", bufs=3))
        attn0 = actx.enter_context(tc.tile_pool(name="attn0", bufs=1))
        attn1 = actx.enter_context(tc.tile_pool(name="attn1", bufs=1))

        # ---- P0: qkv(b0), double-buffered through psF (conv1 not started) ----
        for u in qkv_units(0, qkp0, psF):
            u()
        # residual prefold (vector work while PE chews on attention)
        for b in range(NB):
            for st in range(ST):
                nc.vector.tensor_add(
                    out=xnbo[:, b, st, :], in0=xnbo[:, b, st, :], in1=bo_sb[:]
                )

        hT0 = hT0p.tile([128, FT, SP], BF16, tag="hT0")
        nc.gpsimd.memset(hT0[:, :, 0:4], 0.0)
        nc.gpsimd.memset(hT0[:, :, 4 + S : SP], 0.0)

        # ---- P1 ----
        A = scoreav_units(0, expp, mskp, smal, attn0)
        Bq = qkv_units(1, qkp1, psF)
        Bmain, Brsv = Bq[:-6], Bq[-6:]
        weave(A, Bmain, frac=0.85)
        # drain AV(b0) vector chains behind the qkv leftovers before outproj
        for uu in Brsv:
            uu()
        C = outproj_units(0, lnp)
        D1 = conv1_units(0, (0,), w1p, psF, hT0)
        for i in range(5):
            C[i]()
        weave(C[5:], D1, frac=0.3)

        # ---- P2 ----
        E = scoreav_units(1, expp, mskp, smal, attn1)
        Fc = conv1_units(0, (1,), w1p, psF, hT0)
        Fmain, Frsv = Fc[:-6], Fc[-6:]
        weave(E, Fmain, frac=0.8)
        G = outproj_units(1, lnp)
        weave(G, Frsv)

        # ---- P3: attention pools released; w2 takes their SBUF ----
        actx.close()
        with ExitStack() as p3:
            w2p = p3.enter_context(tc.tile_pool(name="w2p", bufs=1))
            w2 = w2p.tile([128, K, FT, D], BF16, tag="w2")
            nc.sync.dma_start(w2[:], wc2_d[:])
            hT1p = p3.enter_context(tc.tile_pool(name="hT1p", bufs=1))
            hT1 = hT1p.tile([128, FT, SP], BF16, tag="hT1")
            nc.gpsimd.memset(hT1[:, :, 0:4], 0.0)
            nc.gpsimd.memset(hT1[:, :, 4 + S : SP], 0.0)
            # prefold the very last LN2 residual to shorten the kernel tail
            x1nc7 = persist.tile([128, D], F32, tag="x1nc7")
            nc.vector.tensor_add(
                out=x1nc7[:], in0=x1n[:, 1, ST - 1, :], in1=bc2_sb[:]
            )
            for u in conv1_units(1, (0, 1), w1p, psF, hT1):
                u()
            psG = p3.enter_context(tc.tile_pool(name="psG", bufs=3, space="PSUM"))
            ln2 = p3.enter_context(tc.tile_pool(name="ln2", bufs=2))
            conv2(0, psG, ln2, hT0, w2)
            conv2(1, psG, ln2, hT1, w2, x1nc7=x1nc7)


def _build(trivial_g1, trivial_g2):
    key = ("nc", trivial_g1, trivial_g2)
    if key not in _CACHE:
        nc = bacc.Bacc()
        _CACHE[key] = _emit(nc, trivial_g1, trivial_g2)
    return _CACHE[key]


def _prep_shared(Wq, bq, Wk, bk, Wv, bv, Wo, bo, Wc1, bc1, Wc2, bc2, g1, beta1, g2, beta2):
    bf = ml_dtypes.bfloat16
    f32 = np.float32
    sh = {}
    pm = lambda w: np.ascontiguousarray(w.transpose(2, 0, 1, 3).astype(bf))
    sh["wq"] = pm(Wq.reshape(H, DC, 128, DH))
    sh["wk"] = pm(Wk.reshape(H, DC, 128, DH))
    sh["wv"] = pm(Wv.reshape(H, DC, 128, DH))
    wo = np.zeros((4, 128, D), dtype=bf)
    bounds = ((0, 128), (128, 192), (192, 320), (320, 384))
    for c, (r0, r1) in enumerate(bounds):
        wo[c, : r1 - r0] = Wo[r0:r1].astype(bf)
    sh["wo"] = np.ascontiguousarray(wo.transpose(1, 0, 2))
    sh["wc1"] = pm(Wc1.reshape(K, DC, 128, F))
    sh["wc2"] = pm(Wc2.reshape(K, FT, 128, D))
    bqk = np.zeros((2, H, 2, 128), dtype=f32)
    for i, bb in enumerate((bq, bk)):
        for h in range(H):
            bqk[i, h, 0, :] = bb[h, :128]
            bqk[i, h, 1, :64] = bb[h, 128:]
    sh["bqk"] = np.ascontiguousarray(bqk.transpose(3, 0, 1, 2))
    sh["bv"] = bv.astype(f32)
    sh["bo"] = bo.astype(f32)
    sh["bc1t"] = np.ascontiguousarray(bc1.reshape(FT, 128).T.astype(f32))
    sh["bc2"] = bc2.astype(f32)
    sh["g1"] = g1.astype(f32)
    sh["be1"] = beta1.astype(f32)
    sh["g2"] = g2.astype(f32)
    sh["be2"] = beta2.astype(f32)
    return sh


def run_sharded(inputs, trace=False):
    x = np.asarray(inputs["x"], dtype=np.float32)
    mask = np.asarray(inputs["mask"])
    sh = _prep_shared(
        *[np.asarray(inputs[k]) for k in (
            "Wq", "bq", "Wk", "bk", "Wv", "bv", "Wo", "bo",
            "Wc1", "bc1", "Wc2", "bc2", "g1", "beta1", "g2", "beta2",
        )]
    )
    triv1 = bool(
        np.all(sh["g1"] == 1.0) and np.all(sh["be1"] == 0.0)
    )
    triv2 = bool(
        np.all(sh["g2"] == 1.0) and np.all(sh["be2"] == 0.0)
    )
    nc = _build(triv1, triv2)
    bf = ml_dtypes.bfloat16
    in_maps = []
    for c in range(NCORES):
        xb = x[c * NB : (c + 1) * NB]  # [NB, S, D]
        m = {}
        m["xT"] = np.ascontiguousarray(xb.transpose(0, 2, 1)).reshape(NB, DC, 128, S).astype(bf)
        m["xn"] = np.ascontiguousarray(xb.reshape(NB, ST, 128, D)).astype(bf)
        mb = mask[c * NB : (c + 1) * NB]
        m["mT"] = np.ascontiguousarray(
            (~mb.transpose(0, 2, 1)).astype(bf)
        ).reshape(NB, ST, 128, S)
        m.update(sh)
        in_maps.append(m)
    res = run_bass_kernel_spmd(nc, in_maps, core_ids=list(range(NCORES)), trace=trace)
    out = np.empty((B, S, D), dtype=np.float32)
    for c in range(NCORES):
        out[c * NB : (c + 1) * NB] = res.results[c]["y"].reshape(NB, S, D)
    return out, res


def kernel(**inputs):
    out, _ = run_sharded(inputs, trace=False)
    return out



# revision 8
# speedup vs baseline: 1.0087x; 1.0087x over previous
"""FFTBlock (attention + conv-FFN transformer block) on 8 Trainium2 NeuronCores.

Data-parallel over batch: 16 batch items -> 2 per core. Each core runs the
full block (MHA + LN + conv1d-FFN + LN) on its 2 batch items.

Schedule (per core): a single woven PE stream designed so the tensor engine
never idles long enough for the HAM clock gate to re-throttle:

  P0: qkv(b0)                               (weights host-packed partition-major
                                             so every weight DMA is rectangular)
  P1: attn-scores/AV(b0)  ~weave~ qkv(b1)
      outproj(b0)         ~weave~ conv1(b0, qc=0)   <- dense PE filler
  P2: attn-scores/AV(b1)  ~weave~ conv1(b0, qc=1)
      outproj(b1)         ~weave~ conv1(b0) leftovers
  P3: conv1(b1), conv2(b0), conv2(b1)       (w2 prefetched at P3 start)

Other structural points:
  - residual (xn + bo) precomputed into SBUF off the critical path; LN chains
    use bn_stats/bn_aggr + Rsqrt; gamma/beta applied only if nontrivial.
  - scores matmuls zero-pad the DH=192 contraction to 2x128 partitions
    (64-partition matmuls measure ~1.5x slower than 128).
  - x1n (post-LN1) stored bf16 so PE transposes for the conv input run at
    1 cycle/row; softmax denominator via ones-column in V.
  - PSUM statically planned at exactly 8 banks.
"""

import sys

sys.path.insert(0, "/opt/trn_rl_repo")

import math
from contextlib import ExitStack

import ml_dtypes
import numpy as np

import concourse.bass as bass
import concourse.mybir as mybir
import concourse.tile as tile
from concourse import bacc
from concourse.bass_utils import run_bass_kernel_spmd
from concourse.masks import make_identity

BF16 = mybir.dt.bfloat16
F32 = mybir.dt.float32
AF = mybir.ActivationFunctionType
ALU = mybir.AluOpType

B, S, D, H, DH, F, K = 16, 1024, 384, 2, 192, 1536, 9
NCORES = 8
NB = B // NCORES  # batch items per core
EPS = 1e-5
ISCALE = 1.0 / math.sqrt(D)  # NOTE: reference scales by sqrt(d_model)
SP = S + 8  # padded sequence length (4 left, 4 right)
DC = D // 128  # 3 d-chunks
FT = F // 128  # 12 filter tiles
ST = S // 128  # 8 seq tiles of 128
SQ = S // 512  # 2 seq chunks of 512

_CACHE = {}


def _bcast(ap, p=128):
    return bass.AP(tensor=ap.tensor, offset=ap.offset, ap=[[0, p]] + list(ap.ap))


def weave(a, b, frac=1.0):
    # proportional merge of two unit lists; emits every closure.
    # `frac`: a is fully emitted once frac of b has been emitted, so the tail
    # of b covers a's trailing cross-engine latency with dense PE work.
    nb = max(1, int(len(b) * frac))
    ia = ib = 0
    while ia < len(a) or ib < len(b):
        if ib >= len(b) or (ia < len(a) and ia * (nb + 1) <= ib * (len(a) + 1)):
            a[ia]()
            ia += 1
        else:
            b[ib]()
            ib += 1


def _emit(nc, trivial_g1, trivial_g2):
    # ---- DRAM I/O (all weights host-packed partition-major) ----
    d = {}
    d["xT_d"] = nc.dram_tensor("xT", [NB, DC, 128, S], BF16, kind="ExternalInput")
    d["xn_d"] = nc.dram_tensor("xn", [NB, ST, 128, D], BF16, kind="ExternalInput")
    d["mT_d"] = nc.dram_tensor("mT", [NB, ST, 128, S], BF16, kind="ExternalInput")
    d["wq_d"] = nc.dram_tensor("wq", [128, H, DC, DH], BF16, kind="ExternalInput")
    d["wk_d"] = nc.dram_tensor("wk", [128, H, DC, DH], BF16, kind="ExternalInput")
    d["wv_d"] = nc.dram_tensor("wv", [128, H, DC, DH], BF16, kind="ExternalInput")
    d["wo_d"] = nc.dram_tensor("wo", [128, 4, D], BF16, kind="ExternalInput")
    d["wc1_d"] = nc.dram_tensor("wc1", [128, K, DC, F], BF16, kind="ExternalInput")
    d["wc2_d"] = nc.dram_tensor("wc2", [128, K, FT, D], BF16, kind="ExternalInput")
    d["bqk_d"] = nc.dram_tensor("bqk", [128, 2, H, 2], F32, kind="ExternalInput")
    d["bv_d"] = nc.dram_tensor("bv", [H, DH], F32, kind="ExternalInput")
    d["bo_d"] = nc.dram_tensor("bo", [D], F32, kind="ExternalInput")
    d["bc1_d"] = nc.dram_tensor("bc1t", [128, FT], F32, kind="ExternalInput")
    d["bc2_d"] = nc.dram_tensor("bc2", [D], F32, kind="ExternalInput")
    d["g1_d"] = nc.dram_tensor("g1", [D], F32, kind="ExternalInput")
    d["be1_d"] = nc.dram_tensor("be1", [D], F32, kind="ExternalInput")
    d["g2_d"] = nc.dram_tensor("g2", [D], F32, kind="ExternalInput")
    d["be2_d"] = nc.dram_tensor("be2", [D], F32, kind="ExternalInput")
    d["y_d"] = nc.dram_tensor("y", [NB, ST, 128, D], F32, kind="ExternalOutput")
    d["trivial_g1"] = trivial_g1
    d["trivial_g2"] = trivial_g2

    with tile.TileContext(nc) as tc:
        _body(nc, tc, d)
    nc.finalize()
    return nc


def _body(nc, tc, d):
    xT_d, xn_d, mT_d = d["xT_d"], d["xn_d"], d["mT_d"]
    wq_d, wk_d, wv_d, wo_d = d["wq_d"], d["wk_d"], d["wv_d"], d["wo_d"]
    wc1_d, wc2_d = d["wc1_d"], d["wc2_d"]
    bqk_d, bv_d, bo_d, bc1_d, bc2_d = (
        d["bqk_d"], d["bv_d"], d["bo_d"], d["bc1_d"], d["bc2_d"],
    )
    g1_d, be1_d, g2_d, be2_d, y_d = d["g1_d"], d["be1_d"], d["g2_d"], d["be2_d"], d["y_d"]
    triv1, triv2 = d["trivial_g1"], d["trivial_g2"]

    with ExitStack() as ctx:
        const = ctx.enter_context(tc.tile_pool(name="const", bufs=1))
        persist = ctx.enter_context(tc.tile_pool(name="persist", bufs=1))

        # ---- long-lived conv pools first (LIFO stack: created before actx) ----
        w1p = ctx.enter_context(tc.tile_pool(name="w1p", bufs=2))
        psF = ctx.enter_context(tc.tile_pool(name="psF", bufs=2, space="PSUM"))
        hT0p = ctx.enter_context(tc.tile_pool(name="hT0p", bufs=1, side="right"))

        # ---- attention-lifetime pools (closed before P3 to fit w2) ----
        actx = ctx.enter_context(ExitStack())

        # ---- critical-path DMAs first: what qkv(b0) needs ----
        xtp = actx.enter_context(tc.tile_pool(name="xtp", bufs=1))
        xT_sb = xtp.tile([128, NB, DC, S], BF16, tag="xT")
        nc.sync.dma_start(xT_sb[:, 0], xT_d[0].rearrange("c p s -> p c s"))
        wq_sb = const.tile([128, H, DC, DH], BF16, tag="wq")
        nc.sync.dma_start(wq_sb[:], wq_d[:])
        wk_sb = const.tile([128, H, DC, DH], BF16, tag="wk")
        nc.sync.dma_start(wk_sb[:], wk_d[:])
        bqk_sb = const.tile([128, 2, H, 2], F32, tag="bqk")
        nc.sync.dma_start(bqk_sb[:], bqk_d[:])
        wv_sb = const.tile([128, H, DC, DH], BF16, tag="wv")
        nc.sync.dma_start(wv_sb[:], wv_d[:])
        bv_sb = const.tile([128, H, DH], F32, tag="bv")
        nc.sync.dma_start(bv_sb[:], _bcast(bv_d[:]))
        nc.sync.dma_start(xT_sb[:, 1], xT_d[1].rearrange("c p s -> p c s"))

        # residual (xn + bo), prefolded off the critical path
        xnp = actx.enter_context(tc.tile_pool(name="xnp", bufs=1))
        xnbo = xnp.tile([128, NB, ST, D], BF16, tag="xnbo")
        nc.sync.dma_start(xnbo[:], xn_d[:].rearrange("b s p d -> p b s d"))
        bo_sb = const.tile([128, D], F32, tag="bo")
        nc.sync.dma_start(bo_sb[:], _bcast(bo_d[:]))

        # remaining constants (off the critical path)
        wo_sb = const.tile([128, 4, D], BF16, tag="wo")
        nc.sync.dma_start(wo_sb[:], wo_d[:])
        ident = const.tile([128, 128], BF16, tag="ident")
        make_identity(nc, ident[:])
        bc1_sb = const.tile([128, FT], F32, tag="bc1")
        nc.sync.dma_start(bc1_sb[:], bc1_d[:])
        bc2_sb = const.tile([128, D], F32, tag="bc2")
        nc.sync.dma_start(bc2_sb[:], _bcast(bc2_d[:]))
        if not triv1:
            g1_sb = const.tile([128, D], F32, tag="g1")
            nc.sync.dma_start(g1_sb[:], _bcast(g1_d[:]))
            be1_sb = const.tile([128, D], F32, tag="be1")
            nc.sync.dma_start(be1_sb[:], _bcast(be1_d[:]))
        if not triv2:
            g2_sb = const.tile([128, D], F32, tag="g2")
            nc.sync.dma_start(g2_sb[:], _bcast(g2_d[:]))
            be2_sb = const.tile([128, D], F32, tag="be2")
            nc.sync.dma_start(be2_sb[:], _bcast(be2_d[:]))
        eps_sb = const.tile([128, 1], F32, tag="eps")
        nc.vector.memset(eps_sb[:], EPS)

        x1T = persist.tile([128, NB, DC, SP], BF16, tag="x1T")
        x1n = persist.tile([128, NB, ST, D], BF16, tag="x1n")
        for b in range(NB):
            nc.gpsimd.memset(x1T[:, b, :, 0:4], 0.0)
            nc.gpsimd.memset(x1T[:, b, :, 4 + S : SP], 0.0)

        # ---- global PSUM plan: exactly 8 banks ----
        # psF(2): conv1 + all qkv | psS(1) scores | av0(1)+av1(1)
        # psP(1) outproj | psT(2) transposes
        psS = actx.enter_context(tc.tile_pool(name="psS", bufs=1, space="PSUM"))
        psAV = actx.enter_context(tc.tile_pool(name="psAV", bufs=1, space="PSUM"))
        psP = actx.enter_context(tc.tile_pool(name="psP", bufs=1, space="PSUM"))
        psT = actx.enter_context(tc.tile_pool(name="psT", bufs=2, space="PSUM"))

        QT, KT, VV, ON = {}, {}, {}, {}

        def qkv_units(b, qkp, pspool):
            units = []
            for h in range(H):
                qt = qkp.tile([128, 2, S], BF16, tag=f"qt{b}{h}")
                kt = qkp.tile([128, 2, S], BF16, tag=f"kt{b}{h}")
                vv = qkp.tile([128, ST, DH + 1], BF16, tag=f"vv{b}{h}")
                QT[b, h], KT[b, h], VV[b, h] = qt, kt, vv
                # zero-pad rows 64:128 of the second DH chunk so scores can
                # run full-128-partition contractions
                nc.gpsimd.memset(qt[64:128, 1, :], 0.0)
                nc.gpsimd.memset(kt[64:128, 1, :], 0.0)
                for wsb, bi, dst in ((wq_sb, 0, qt), (wk_sb, 1, kt)):
                    for mc, (m0, msz) in enumerate(((0, 128), (128, 64))):
                        for qc in range(SQ):
                            def u(b=b, h=h, wsb=wsb, bi=bi, dst=dst, m0=m0,
                                  msz=msz, mc=mc, qc=qc):
                                ps = pspool.tile([128, 512], F32, tag="c1")
                                for dc in range(DC):
                                    nc.tensor.matmul(
                                        ps[:msz, :],
                                        lhsT=wsb[:, h, dc, m0 : m0 + msz],
                                        rhs=xT_sb[:, b, dc, qc * 512 : qc * 512 + 512],
                                        start=(dc == 0),
                                        stop=(dc == DC - 1),
                                    )
                                nc.scalar.activation(
                                    out=dst[:msz, mc, qc * 512 : qc * 512 + 512],
                                    in_=ps[:msz, :],
                                    func=AF.Identity,
                                    bias=bqk_sb[:msz, bi, h, mc : mc + 1],
                                    scale=1.0,
                                )
                            units.append(u)
                for st in range(ST):
                    def u(b=b, h=h, vv=vv, st=st):
                        ps = pspool.tile([128, 512], F32, tag="c1")
                        for dc in range(DC):
                            nc.tensor.matmul(
                                ps[:, :DH],
                                lhsT=xT_sb[:, b, dc, st * 128 : st * 128 + 128],
                                rhs=wv_sb[:, h, dc, :],
                                start=(dc == 0),
                                stop=(dc == DC - 1),
                            )
                        nc.vector.tensor_add(
                            out=vv[:, st, 0:DH], in0=ps[:, :DH], in1=bv_sb[:, h, :]
                        )
                        nc.gpsimd.memset(vv[:, st, DH : DH + 1], 1.0)
                    units.append(u)
            return units

        def scoreav_units(b, expp, mskp, smal, attn):
            units = []
            for h in range(H):
                expT = expp.tile([128, ST, S], BF16, tag="expT")
                mtiles = {}
                # prefetch the first masks of this head (deep prefetch: a
                # [128,1024] bf16 mask tile is ~256KB and takes several us on
                # one DMA queue; shallow prefetch stalled the PE in P1/P2)
                def mhead(b=b, mtiles=mtiles):
                    for kc in range(4):
                        mt = mskp.tile([128, S], BF16, tag="mt")
                        mtiles[kc] = mt
                        nc.sync.dma_start(mt[:], mT_d[b, kc])
                units.append(mhead)
                for kc in range(ST):
                    for qc in range(SQ):
                        def u(b=b, h=h, expT=expT, kc=kc, qc=qc, mtiles=mtiles):
                            qt, kt = QT[b, h], KT[b, h]
                            qs = slice(qc * 512, qc * 512 + 512)
                            ps = psS.tile([128, 512], F32, tag="sc")
                            for mc in range(2):
                                nc.tensor.matmul(
                                    ps[:],
                                    lhsT=kt[:, mc, kc * 128 : kc * 128 + 128],
                                    rhs=qt[:, mc, qs],
                                    start=(mc == 0),
                                    stop=(mc == 1),
                                )
                            nc.scalar.activation(
                                out=expT[:, kc, qs], in_=ps[:], func=AF.Exp,
                                scale=ISCALE,
                            )
                            if qc == 0 and kc + 4 < ST:
                                mt = mskp.tile([128, S], BF16, tag="mt")
                                mtiles[kc + 4] = mt
                                nc.sync.dma_start(mt[:], mT_d[b, kc + 4])
                            nc.vector.tensor_mul(
                                out=expT[:, kc, qs], in0=expT[:, kc, qs],
                                in1=mtiles[kc][:, qs],
                            )
                        units.append(u)
                onrm = attn.tile([128, 2, S], BF16, tag=f"on{b}{h}")
                ON[b, h] = onrm
                for qc in range(SQ):
                    def u(b=b, h=h, expT=expT, onrm=onrm, qc=qc):
                        vv = VV[b, h]
                        qs = slice(qc * 512, qc * 512 + 512)
                        ps0 = psAV.tile([128, 512], F32, tag="av0")
                        ps1 = psAV.tile([65, 512], F32, tag="av1")
                        for kc in range(ST):
                            nc.tensor.matmul(
                                ps0[:],
                                lhsT=vv[:, kc, 0:128],
                                rhs=expT[:, kc, qs],
                                start=(kc == 0),
                                stop=(kc == ST - 1),
                            )
                            nc.tensor.matmul(
                                ps1[:],
                                lhsT=vv[:, kc, 128 : DH + 1],
                                rhs=expT[:, kc, qs],
                                start=(kc == 0),
                                stop=(kc == ST - 1),
                            )
                        rc = smal.tile([1, 512], F32, tag="rc")
                        nc.vector.reciprocal(rc[:], ps1[64:65, :])
                        rb = smal.tile([128, 512], F32, tag="rb")
                        nc.gpsimd.partition_broadcast(rb[:], rc[:])
                        nc.vector.tensor_mul(out=onrm[:, 0, qs], in0=ps0[:], in1=rb[:])
                        nc.vector.tensor_mul(
                            out=onrm[:64, 1, qs], in0=ps1[:64, :], in1=rb[:64, :]
                        )
                    units.append(u)
            return units

        chunks = ((0, 0, 128, 0), (0, 1, 64, 1), (1, 0, 128, 2), (1, 1, 64, 3))

        def outproj_units(b, lnp):
            units = []
            for st in range(ST):
                def u(b=b, st=st):
                    ps = psP.tile([128, D], F32, tag="op")
                    for i, (h, c, ksz, wc) in enumerate(chunks):
                        nc.tensor.matmul(
                            ps[:],
                            lhsT=ON[b, h][:ksz, c, st * 128 : st * 128 + 128],
                            rhs=wo_sb[:ksz, wc, :],
                            start=(i == 0),
                            stop=(i == 3),
                        )
                    t = lnp.tile([128, D], F32, tag="t")
                    nc.vector.tensor_add(out=t[:], in0=ps[:], in1=xnbo[:, b, st, :])
                    stats = lnp.tile([128, 6], F32, tag="st")
                    nc.vector.bn_stats(out=stats[:], in_=t[:])
                    mv = lnp.tile([128, 2], F32, tag="mv")
                    nc.vector.bn_aggr(out=mv[:], in_=stats[:])
                    sd = lnp.tile([128, 1], F32, tag="sd")
                    nc.scalar.activation(
                        out=sd[:], in_=mv[:, 1:2], func=AF.Sqrt, bias=eps_sb[:],
                    )
                    nc.vector.reciprocal(sd[:], sd[:])
                    xv = x1n[:, b, st, :]
                    nc.vector.tensor_scalar(
                        out=xv, in0=t[:], scalar1=mv[:, 0:1], scalar2=sd[:],
                        op0=ALU.subtract, op1=ALU.mult,
                    )
                    if not triv1:
                        nc.gpsimd.tensor_mul(out=xv, in0=xv, in1=g1_sb[:])
                        nc.gpsimd.tensor_add(out=xv, in0=xv, in1=be1_sb[:])
                    for dc in range(DC):
                        tp = psT.tile([128, 128], BF16, tag="tp")
                        nc.tensor.transpose(
                            tp[:], x1n[:, b, st, dc * 128 : dc * 128 + 128], ident[:]
                        )
                        nc.scalar.copy(
                            out=x1T[:, b, dc, 4 + st * 128 : 4 + st * 128 + 128],
                            in_=tp[:],
                        )
                units.append(u)
            return units

        def conv1_units(b, qcs, w1p, psF, hT, state=None):
            # `state` may arrive pre-seeded with ft=0's w1 tile (prefetched
            # during P2's tail so P3's first matmuls don't stall on DMA)
            units = []
            skip_first = state is not None and "w1" in state
            if state is None:
                state = {}
            for ft in range(FT):
                if not (ft == 0 and skip_first):
                    def udma(ft=ft, state=state):
                        w1 = w1p.tile([128, K, DC, 128], BF16, tag="w1")
                        state["w1"] = w1
                        nc.sync.dma_start(w1[:], wc1_d[:, :, :, ft * 128 : ft * 128 + 128])
                    units.append(udma)
                for qc in qcs:
                    def u(b=b, ft=ft, qc=qc, state=state):
                        w1 = state["w1"]
                        ps = psF.tile([128, 512], F32, tag="c1")
                        idx = 0
                        for k9 in range(K):
                            for dc in range(DC):
                                nc.tensor.matmul(
                                    ps[:],
                                    lhsT=w1[:, k9, dc, :],
                                    rhs=x1T[:, b, dc, qc * 512 + k9 : qc * 512 + k9 + 512],
                                    start=(idx == 0),
                                    stop=(idx == K * DC - 1),
                                )
                                idx += 1
                        nc.scalar.activation(
                            out=hT[:, ft, 4 + qc * 512 : 4 + qc * 512 + 512],
                            in_=ps[:],
                            func=AF.Relu,
                            bias=bc1_sb[:, ft : ft + 1],
                            scale=1.0,
                        )
                    units.append(u)
            return units

        def conv2(b, psG, ln2, hT, w2, x1nc7=None):
            for st in range(ST):
                ps = psG.tile([128, D], F32, tag="c2")
                idx = 0
                for k9 in range(K):
                    for fc in range(FT):
                        nc.tensor.matmul(
                            ps[:],
                            lhsT=hT[:, fc, st * 128 + k9 : st * 128 + k9 + 128],
                            rhs=w2[:, k9, fc, :],
                            start=(idx == 0),
                            stop=(idx == K * FT - 1),
                        )
                        idx += 1
                t = ln2.tile([128, D], F32, tag="t")
                if x1nc7 is not None and st == ST - 1:
                    nc.vector.tensor_add(out=t[:], in0=ps[:], in1=x1nc7[:])
                else:
                    nc.vector.tensor_add(out=t[:], in0=ps[:], in1=x1n[:, b, st, :])
                    nc.vector.tensor_add(out=t[:], in0=t[:], in1=bc2_sb[:])
                stats = ln2.tile([128, 6], F32, tag="st")
                nc.vector.bn_stats(out=stats[:], in_=t[:])
                mv = ln2.tile([128, 2], F32, tag="mv")
                nc.vector.bn_aggr(out=mv[:], in_=stats[:])
                sd = ln2.tile([128, 1], F32, tag="sd")
                nc.scalar.activation(
                    out=sd[:], in_=mv[:, 1:2], func=AF.Sqrt, bias=eps_sb[:],
                )
                nc.vector.reciprocal(sd[:], sd[:])
                ot = ln2.tile([128, D], F32, tag="o")
                nc.vector.tensor_scalar(
                    out=ot[:], in0=t[:], scalar1=mv[:, 0:1], scalar2=sd[:],
                    op0=ALU.subtract, op1=ALU.mult,
                )
                if not triv2:
                    nc.vector.tensor_mul(out=ot[:], in0=ot[:], in1=g2_sb[:])
                    nc.vector.tensor_add(out=ot[:], in0=ot[:], in1=be2_sb[:])
                nc.sync.dma_start(y_d[b, st], ot[:])

        # ================== emission ==================
        qkp0 = actx.enter_context(tc.tile_pool(name="qkp0", bufs=1))
        qkp1 = actx.enter_context(tc.tile_pool(name="qkp1", bufs=1))
        expp = actx.enter_context(tc.tile_pool(name="expp", bufs=1))
        mskp = actx.enter_context(tc.tile_pool(name="mskp", bufs=6))
        smal = actx.enter_context(tc.tile_pool(name="smal", bufs=2))
        lnp = actx.enter_context(tc.tile_pool(name="lnp", bufs=3))
        attn0 = actx.enter_context(tc.tile_pool(name="attn0", bufs=1))
        attn1 = actx.enter_context(tc.tile_pool(name="attn1", bufs=1))

        # ---- P0: qkv(b0), double-buffered through psF (conv1 not started) ----
        for u in qkv_units(0, qkp0, psF):
            u()
        # residual prefold (vector work while PE chews on attention)
        for b in range(NB):
            for st in range(ST):
                nc.vector.tensor_add(
                    out=xnbo[:, b, st, :], in0=xnbo[:, b, st, :], in1=bo_sb[:]
                )

        hT0 = hT0p.tile([128, FT, SP], BF16, tag="hT0")
        nc.gpsimd.memset(hT0[:, :, 0:4], 0.0)
        nc.gpsimd.memset(hT0[:, :, 4 + S : SP], 0.0)

        # ---- P1 ----
        A = scoreav_units(0, expp, mskp, smal, attn0)
        Bq = qkv_units(1, qkp1, psF)
        Bmain, Brsv = Bq[:-6], Bq[-6:]
        weave(A, Bmain, frac=0.85)
        # drain AV(b0) vector chains behind the qkv leftovers before outproj
        for uu in Brsv:
            uu()
        C = outproj_units(0, lnp)
        D1 = conv1_units(0, (0,), w1p, psF, hT0)
        for i in range(5):
            C[i]()
        weave(C[5:], D1, frac=0.3)

        # ---- P2 ----
        E = scoreav_units(1, expp, mskp, smal, attn1)
        Fc = conv1_units(0, (1,), w1p, psF, hT0)
        Fmain, Frsv = Fc[:-6], Fc[-6:]
        weave(E, Fmain, frac=0.8)
        G = outproj_units(1, lnp)
        weave(G, Frsv)
        # prefetch P3's first w1 tile while outproj(b1) still feeds the PE
        p3w1 = {}
        w1pf = w1p.tile([128, K, DC, 128], BF16, tag="w1")
        p3w1["w1"] = w1pf
        nc.sync.dma_start(w1pf[:], wc1_d[:, :, :, 0:128])

        # ---- P3: attention pools released; w2 takes their SBUF ----
        actx.close()
        with ExitStack() as p3:
            w2p = p3.enter_context(tc.tile_pool(name="w2p", bufs=1))
            w2 = w2p.tile([128, K, FT, D], BF16, tag="w2")
            nc.sync.dma_start(w2[:], wc2_d[:])
            hT1p = p3.enter_context(tc.tile_pool(name="hT1p", bufs=1))
            hT1 = hT1p.tile([128, FT, SP], BF16, tag="hT1")
            nc.gpsimd.memset(hT1[:, :, 0:4], 0.0)
            nc.gpsimd.memset(hT1[:, :, 4 + S : SP], 0.0)
            # prefold the very last LN2 residual to shorten the kernel tail
            x1nc7 = persist.tile([128, D], F32, tag="x1nc7")
            nc.vector.tensor_add(
                out=x1nc7[:], in0=x1n[:, 1, ST - 1, :], in1=bc2_sb[:]
            )
            for u in conv1_units(1, (0, 1), w1p, psF, hT1, state=p3w1):
                u()
            psG = p3.enter_context(tc.tile_pool(name="psG", bufs=3, space="PSUM"))
            ln2 = p3.enter_context(tc.tile_pool(name="ln2", bufs=2))
            conv2(0, psG, ln2, hT0, w2)
            conv2(1, psG, ln2, hT1, w2, x1nc7=x1nc7)


def _build(trivial_g1, trivial_g2):
    key = ("nc", trivial_g1, trivial_g2)
    if key not in _CACHE:
        nc = bacc.Bacc()
        _CACHE[key] = _emit(nc, trivial_g1, trivial_g2)
    return _CACHE[key]


def _prep_shared(Wq, bq, Wk, bk, Wv, bv, Wo, bo, Wc1, bc1, Wc2, bc2, g1, beta1, g2, beta2):
    bf = ml_dtypes.bfloat16
    f32 = np.float32
    sh = {}
    pm = lambda w: np.ascontiguousarray(w.transpose(2, 0, 1, 3).astype(bf))
    sh["wq"] = pm(Wq.reshape(H, DC, 128, DH))
    sh["wk"] = pm(Wk.reshape(H, DC, 128, DH))
    sh["wv"] = pm(Wv.reshape(H, DC, 128, DH))
    wo = np.zeros((4, 128, D), dtype=bf)
    bounds = ((0, 128), (128, 192), (192, 320), (320, 384))
    for c, (r0, r1) in enumerate(bounds):
        wo[c, : r1 - r0] = Wo[r0:r1].astype(bf)
    sh["wo"] = np.ascontiguousarray(wo.transpose(1, 0, 2))
    sh["wc1"] = pm(Wc1.reshape(K, DC, 128, F))
    sh["wc2"] = pm(Wc2.reshape(K, FT, 128, D))
    bqk = np.zeros((2, H, 2, 128), dtype=f32)
    for i, bb in enumerate((bq, bk)):
        for h in range(H):
            bqk[i, h, 0, :] = bb[h, :128]
            bqk[i, h, 1, :64] = bb[h, 128:]
    sh["bqk"] = np.ascontiguousarray(bqk.transpose(3, 0, 1, 2))
    sh["bv"] = bv.astype(f32)
    sh["bo"] = bo.astype(f32)
    sh["bc1t"] = np.ascontiguousarray(bc1.reshape(FT, 128).T.astype(f32))
    sh["bc2"] = bc2.astype(f32)
    sh["g1"] = g1.astype(f32)
    sh["be1"] = beta1.astype(f32)
    sh["g2"] = g2.astype(f32)
    sh["be2"] = beta2.astype(f32)
    return sh


def run_sharded(inputs, trace=False):
    x = np.asarray(inputs["x"], dtype=np.float32)
    mask = np.asarray(inputs["mask"])
    sh = _prep_shared(
        *[np.asarray(inputs[k]) for k in (
            "Wq", "bq", "Wk", "bk", "Wv", "bv", "Wo", "bo",
            "Wc1", "bc1", "Wc2", "bc2", "g1", "beta1", "g2", "beta2",
        )]
    )
    triv1 = bool(
        np.all(sh["g1"] == 1.0) and np.all(sh["be1"] == 0.0)
    )
    triv2 = bool(
        np.all(sh["g2"] == 1.0) and np.all(sh["be2"] == 0.0)
    )
    nc = _build(triv1, triv2)
    bf = ml_dtypes.bfloat16
    in_maps = []
    for c in range(NCORES):
        xb = x[c * NB : (c + 1) * NB]  # [NB, S, D]
        m = {}
        m["xT"] = np.ascontiguousarray(xb.transpose(0, 2, 1)).reshape(NB, DC, 128, S).astype(bf)
        m["xn"] = np.ascontiguousarray(xb.reshape(NB, ST, 128, D)).astype(bf)
        mb = mask[c * NB : (c + 1) * NB]
        m["mT"] = np.ascontiguousarray(
            (~mb.transpose(0, 2, 1)).astype(bf)
        ).reshape(NB, ST, 128, S)
        m.update(sh)
        in_maps.append(m)
    res = run_bass_kernel_spmd(nc, in_maps, core_ids=list(range(NCORES)), trace=trace)
    out = np.empty((B, S, D), dtype=np.float32)
    for c in range(NCORES):
        out[c * NB : (c + 1) * NB] = res.results[c]["y"].reshape(NB, S, D)
    return out, res


def kernel(**inputs):
    out, _ = run_sharded(inputs, trace=False)
    return out



# revision 20
# speedup vs baseline: 1.2192x; 1.2087x over previous
"""FFTBlock (attention + conv-FFN transformer block) on 8 Trainium2 NeuronCores.

Data-parallel over batch: 16 batch items -> 2 per core. Each core runs the
full block (MHA + LN + conv1d-FFN + LN) on its 2 batch items.

Schedule (per core): a single woven PE stream designed so the tensor engine
never idles long enough for the HAM clock gate to re-throttle:

  P0: qkv(b0)                               (weights host-packed partition-major
                                             so every weight DMA is rectangular)
  P1: attn-scores/AV(b0)  ~weave~ qkv(b1)
      outproj(b0)         ~weave~ conv1(b0, qc=0)   <- dense PE filler
  P2: attn-scores/AV(b1)  ~weave~ conv1(b0, qc=1)
      outproj(b1)         ~weave~ conv1(b0) leftovers
  P3: conv1(b1), conv2(b0), conv2(b1)       (w2 prefetched at P3 start)

Other structural points:
  - residual (xn + bo) precomputed into SBUF off the critical path; LN chains
    use bn_stats/bn_aggr + Rsqrt; gamma/beta applied only if nontrivial.
  - scores matmuls zero-pad the DH=192 contraction to 2x128 partitions
    (64-partition matmuls measure ~1.5x slower than 128).
  - x1n (post-LN1) stored bf16 so PE transposes for the conv input run at
    1 cycle/row; softmax denominator via ones-column in V.
  - PSUM statically planned at exactly 8 banks.
"""

import sys

sys.path.insert(0, "/opt/trn_rl_repo")

import math
from contextlib import ExitStack

import ml_dtypes
import numpy as np

import concourse.bass as bass
import concourse.mybir as mybir
import concourse.tile as tile
from concourse import bacc
from concourse.bass_utils import run_bass_kernel_spmd
from concourse.masks import make_identity

BF16 = mybir.dt.bfloat16
F8 = mybir.dt.float8e4
F32 = mybir.dt.float32
AF = mybir.ActivationFunctionType
ALU = mybir.AluOpType
DRMODE = mybir.MatmulPerfMode.DoubleRow

B, S, D, H, DH, F, K = 16, 1024, 384, 2, 192, 1536, 9
NCORES = 8
NB = B // NCORES  # batch items per core
EPS = 1e-5
ISCALE = 1.0 / math.sqrt(D)  # NOTE: reference scales by sqrt(d_model)

# fp8 (e4m3) attention scales. Attention output is ~50x smaller than the
# residual, so e4m3 noise in the whole attention path costs <1e-3 rel err
# (measured 4.5e-4 in simulation). All values stay < 150 (e4m3 max 240).
SXT = 32.0    # x (attention input, transposed copy only)
SW = 1024.0   # Wq/Wk/Wv weights
SQK = 32.0    # Q/K tiles
SA = 64.0     # exp'd scores (<= 64*~1.9)
SVQ = 64.0    # V tiles / normalized attention output
SWO = 1024.0  # Wo
CRES = SVQ * SWO  # outproj psum scale; LN1 is scale-invariant so the
                  # residual (xn + bo') is pre-scaled by CRES host-side
SP = S + 8  # padded sequence length (4 left, 4 right)
DC = D // 128  # 3 d-chunks
FT = F // 128  # 12 filter tiles
ST = S // 128  # 8 seq tiles of 128
SQ = S // 512  # 2 seq chunks of 512

_CACHE = {}


def _bcast(ap, p=128):
    return bass.AP(tensor=ap.tensor, offset=ap.offset, ap=[[0, p]] + list(ap.ap))


def weave(a, b, frac=1.0):
    # proportional merge of two unit lists; emits every closure.
    # `frac`: a is fully emitted once frac of b has been emitted, so the tail
    # of b covers a's trailing cross-engine latency with dense PE work.
    nb = max(1, int(len(b) * frac))
    ia = ib = 0
    while ia < len(a) or ib < len(b):
        if ib >= len(b) or (ia < len(a) and ia * (nb + 1) <= ib * (len(a) + 1)):
            a[ia]()
            ia += 1
        else:
            b[ib]()
            ib += 1


def _emit(nc, trivial_g1, trivial_g2):
    # ---- DRAM I/O (all weights host-packed partition-major) ----
    d = {}
    d["xT_d"] = nc.dram_tensor("xT", [NB, DC, 128, S], F8, kind="ExternalInput")
    d["xn_d"] = nc.dram_tensor("xn", [NB, ST, 128, D], BF16, kind="ExternalInput")
    d["mT_d"] = nc.dram_tensor("mT", [NB, ST, 128, S], BF16, kind="ExternalInput")
    d["wq_d"] = nc.dram_tensor("wq", [128, H, DC, DH], F8, kind="ExternalInput")
    d["wk_d"] = nc.dram_tensor("wk", [128, H, DC, DH], F8, kind="ExternalInput")
    d["wv_d"] = nc.dram_tensor("wv", [128, H, DC, DH], F8, kind="ExternalInput")
    d["wo_d"] = nc.dram_tensor("wo", [128, 4, D], F8, kind="ExternalInput")
    d["wc1_d"] = nc.dram_tensor("wc1", [128, K, DC, F], BF16, kind="ExternalInput")
    d["wc2_d"] = nc.dram_tensor("wc2", [128, K, FT, D], BF16, kind="ExternalInput")
    d["bqk_d"] = nc.dram_tensor("bqk", [128, 2, H, 2], F32, kind="ExternalInput")
    d["bo_d"] = nc.dram_tensor("bo", [D], F32, kind="ExternalInput")
    d["bc1_d"] = nc.dram_tensor("bc1t", [128, FT], F32, kind="ExternalInput")
    d["bc2_d"] = nc.dram_tensor("bc2", [D], F32, kind="ExternalInput")
    d["g1_d"] = nc.dram_tensor("g1", [D], F32, kind="ExternalInput")
    d["be1_d"] = nc.dram_tensor("be1", [D], F32, kind="ExternalInput")
    d["g2_d"] = nc.dram_tensor("g2", [D], F32, kind="ExternalInput")
    d["be2_d"] = nc.dram_tensor("be2", [D], F32, kind="ExternalInput")
    d["y_d"] = nc.dram_tensor("y", [NB, ST, 128, D], F32, kind="ExternalOutput")
    d["trivial_g1"] = trivial_g1
    d["trivial_g2"] = trivial_g2

    with tile.TileContext(nc) as tc:
        _body(nc, tc, d)
    nc.finalize()
    return nc


def _body(nc, tc, d):
    xT_d, xn_d, mT_d = d["xT_d"], d["xn_d"], d["mT_d"]
    wq_d, wk_d, wv_d, wo_d = d["wq_d"], d["wk_d"], d["wv_d"], d["wo_d"]
    wc1_d, wc2_d = d["wc1_d"], d["wc2_d"]
    bqk_d, bo_d, bc1_d, bc2_d = (
        d["bqk_d"], d["bo_d"], d["bc1_d"], d["bc2_d"],
    )
    g1_d, be1_d, g2_d, be2_d, y_d = d["g1_d"], d["be1_d"], d["g2_d"], d["be2_d"], d["y_d"]
    triv1, triv2 = d["trivial_g1"], d["trivial_g2"]

    with ExitStack() as ctx:
        const = ctx.enter_context(tc.tile_pool(name="const", bufs=1))
        persist = ctx.enter_context(tc.tile_pool(name="persist", bufs=1))

        # ---- long-lived conv pools first (LIFO stack: created before actx) ----
        w1p = ctx.enter_context(tc.tile_pool(name="w1p", bufs=2))
        psF = ctx.enter_context(tc.tile_pool(name="psF", bufs=2, space="PSUM"))
        hT0p = ctx.enter_context(tc.tile_pool(name="hT0p", bufs=1, side="right"))

        # ---- attention-lifetime pools (closed before P3 to fit w2) ----
        actx = ctx.enter_context(ExitStack())

        # ---- critical-path DMAs first: what qkv(b0) needs ----
        xtp = actx.enter_context(tc.tile_pool(name="xtp", bufs=1))
        xT_sb = xtp.tile([128, NB, DC, S], F8, tag="xT")
        nc.sync.dma_start(xT_sb[:, 0], xT_d[0].rearrange("c p s -> p c s"))
        wq_sb = const.tile([128, H, DC, DH], F8, tag="wq")
        nc.sync.dma_start(wq_sb[:], wq_d[:])
        wk_sb = const.tile([128, H, DC, DH], F8, tag="wk")
        nc.sync.dma_start(wk_sb[:], wk_d[:])
        bqk_sb = const.tile([128, 2, H, 2], F32, tag="bqk")
        nc.sync.dma_start(bqk_sb[:], bqk_d[:])
        wv_sb = const.tile([128, H, DC, DH], F8, tag="wv")
        nc.sync.dma_start(wv_sb[:], wv_d[:])
        nc.sync.dma_start(xT_sb[:, 1], xT_d[1].rearrange("c p s -> p c s"))

        # residual (xn + bo), prefolded off the critical path
        xnp = actx.enter_context(tc.tile_pool(name="xnp", bufs=1))
        xnbo = xnp.tile([128, NB, ST, D], BF16, tag="xnbo")
        nc.sync.dma_start(xnbo[:], xn_d[:].rearrange("b s p d -> p b s d"))
        bo_sb = const.tile([128, D], F32, tag="bo")
        nc.sync.dma_start(bo_sb[:], _bcast(bo_d[:]))

        # remaining constants (off the critical path)
        wo_sb = const.tile([128, 4, D], F8, tag="wo")
        nc.sync.dma_start(wo_sb[:], wo_d[:])
        ident = const.tile([128, 128], BF16, tag="ident")
        make_identity(nc, ident[:])
        bc1_sb = const.tile([128, FT], F32, tag="bc1")
        nc.sync.dma_start(bc1_sb[:], bc1_d[:])
        bc2_sb = const.tile([128, D], F32, tag="bc2")
        nc.sync.dma_start(bc2_sb[:], _bcast(bc2_d[:]))
        if not triv1:
            g1_sb = const.tile([128, D], F32, tag="g1")
            nc.sync.dma_start(g1_sb[:], _bcast(g1_d[:]))
            be1_sb = const.tile([128, D], F32, tag="be1")
            nc.sync.dma_start(be1_sb[:], _bcast(be1_d[:]))
        if not triv2:
            g2_sb = const.tile([128, D], F32, tag="g2")
            nc.sync.dma_start(g2_sb[:], _bcast(g2_d[:]))
            be2_sb = const.tile([128, D], F32, tag="be2")
            nc.sync.dma_start(be2_sb[:], _bcast(be2_d[:]))
        eps_sb = const.tile([128, 1], F32, tag="eps")
        nc.vector.memset(eps_sb[:], EPS)
        # LN1 runs on CRES-scaled values (scale-invariant apart from eps)
        eps1_sb = const.tile([128, 1], F32, tag="eps1")
        nc.vector.memset(eps1_sb[:], EPS * CRES * CRES)
        # exp bias ln(SA) folds the fp8 score scale into the Exp activation
        lnsa_sb = const.tile([128, 1], F32, tag="lnsa")
        nc.vector.memset(lnsa_sb[:], math.log(SA))

        x1T = persist.tile([128, NB, DC, SP], BF16, tag="x1T")
        x1n = persist.tile([128, NB, ST, D], BF16, tag="x1n")
        for b in range(NB):
            nc.gpsimd.memset(x1T[:, b, :, 0:4], 0.0)
            nc.gpsimd.memset(x1T[:, b, :, 4 + S : SP], 0.0)

        # ---- global PSUM plan: exactly 8 banks ----
        # psF(2): conv1 + all qkv | psS(1) scores | av0(1)+av1(1)
        # psP(1) outproj | psT(2) transposes
        psS = actx.enter_context(tc.tile_pool(name="psS", bufs=1, space="PSUM"))
        psAV = actx.enter_context(tc.tile_pool(name="psAV", bufs=1, space="PSUM"))
        psP = actx.enter_context(tc.tile_pool(name="psP", bufs=1, space="PSUM"))
        psT = actx.enter_context(tc.tile_pool(name="psT", bufs=2, space="PSUM"))

        QT, KT, VV, ON = {}, {}, {}, {}

        QK_SCALE = SQK / (SXT * SW)   # psum -> Q/K fp8 tiles
        V_SCALE = SVQ / (SXT * SW)    # psum -> V fp8 tiles (bv folded into bo')

        def qkv_units(b, qkp, pspool):
            units = []
            for h in range(H):
                qt = qkp.tile([128, 2, S], F8, tag=f"qt{b}{h}")
                kt = qkp.tile([128, 2, S], F8, tag=f"kt{b}{h}")
                # vv free dim padded 193->208 so the kc-pair stride is a
                # multiple of 16B (DoubleRow weight-path requirement)
                vv = qkp.tile([128, ST, 208], F8, tag=f"vv{b}{h}")
                QT[b, h], KT[b, h], VV[b, h] = qt, kt, vv
                # zero-pad rows 64:128 of the second DH chunk so scores can
                # run full-256-contraction DoubleRow matmuls
                nc.gpsimd.memset(qt[64:128, 1, :], 0.0)
                nc.gpsimd.memset(kt[64:128, 1, :], 0.0)
                for wsb, bi, dst in ((wq_sb, 0, qt), (wk_sb, 1, kt)):
                    for mc, (m0, msz) in enumerate(((0, 128), (128, 64))):
                        for qc in range(SQ):
                            def u(b=b, h=h, wsb=wsb, bi=bi, dst=dst, m0=m0,
                                  msz=msz, mc=mc, qc=qc):
                                ps = pspool.tile([128, 512], F32, tag="c1")
                                qs = slice(qc * 512, qc * 512 + 512)
                                nc.tensor.matmul(
                                    ps[:msz, :],
                                    lhsT=wsb[:, h, 0:2, m0 : m0 + msz],
                                    rhs=xT_sb[:, b, 0:2, qs],
                                    start=True, stop=False,
                                    perf_mode=DRMODE, skip_group_check=True,
                                )
                                nc.tensor.matmul(
                                    ps[:msz, :],
                                    lhsT=wsb[:, h, 2, m0 : m0 + msz],
                                    rhs=xT_sb[:, b, 2, qs],
                                    start=False, stop=True,
                                    skip_group_check=True,
                                )
                                nc.scalar.activation(
                                    out=dst[:msz, mc, qs],
                                    in_=ps[:msz, :],
                                    func=AF.Identity,
                                    bias=bqk_sb[:msz, bi, h, mc : mc + 1],
                                    scale=QK_SCALE,
                                )
                            units.append(u)
                for st in range(ST):
                    def u(b=b, h=h, vv=vv, st=st):
                        ps = pspool.tile([128, 512], F32, tag="c1")
                        ss = slice(st * 128, st * 128 + 128)
                        nc.tensor.matmul(
                            ps[:, :DH],
                            lhsT=xT_sb[:, b, 0:2, ss],
                            rhs=wv_sb[:, h, 0:2, :],
                            start=True, stop=False,
                            perf_mode=DRMODE, skip_group_check=True,
                        )
                        nc.tensor.matmul(
                            ps[:, :DH],
                            lhsT=xT_sb[:, b, 2, ss],
                            rhs=wv_sb[:, h, 2, :],
                            start=False, stop=True,
                            skip_group_check=True,
                        )
                        nc.scalar.activation(
                            out=vv[:, st, 0:DH], in_=ps[:, :DH],
                            func=AF.Identity, scale=V_SCALE,
                        )
                        nc.gpsimd.memset(vv[:, st, DH : DH + 1], 1.0)
                    units.append(u)
            return units

        def scoreav_units(b, expp, mskp, smal, attn):
            units = []
            for h in range(H):
                expT = expp.tile([128, ST, S], F8, tag="expT")
                mtiles = {}
                # prefetch the first masks of this head (deep prefetch: a
                # [128,1024] bf16 mask tile is ~256KB and takes several us on
                # one DMA queue; shallow prefetch stalled the PE in P1/P2)
                def mhead(b=b, mtiles=mtiles):
                    for kc in range(4):
                        mt = mskp.tile([128, S], BF16, tag="mt")
                        mtiles[kc] = mt
                        nc.sync.dma_start(mt[:], mT_d[b, kc])
                units.append(mhead)
                for kc in range(ST):
                    for qc in range(SQ):
                        def u(b=b, h=h, expT=expT, kc=kc, qc=qc, mtiles=mtiles):
                            qt, kt = QT[b, h], KT[b, h]
                            qs = slice(qc * 512, qc * 512 + 512)
                            ps = psS.tile([128, 512], F32, tag="sc")
                            nc.tensor.matmul(
                                ps[:],
                                lhsT=kt[:, 0:2, kc * 128 : kc * 128 + 128],
                                rhs=qt[:, 0:2, qs],
                                start=True, stop=True,
                                perf_mode=DRMODE,
                            )
                            nc.scalar.activation(
                                out=expT[:, kc, qs], in_=ps[:], func=AF.Exp,
                                scale=ISCALE / (SQK * SQK), bias=lnsa_sb[:],
                            )
                            if qc == 0 and kc + 4 < ST:
                                mt = mskp.tile([128, S], BF16, tag="mt")
                                mtiles[kc + 4] = mt
                                nc.sync.dma_start(mt[:], mT_d[b, kc + 4])
                            nc.vector.tensor_mul(
                                out=expT[:, kc, qs], in0=expT[:, kc, qs],
                                in1=mtiles[kc][:, qs],
                            )
                        units.append(u)
                onrm = attn.tile([128, 2, S], F8, tag=f"on{b}{h}")
                ON[b, h] = onrm
                # rows 64:128 of the odd DH chunk feed full-128 DoubleRow
                # contractions in outproj; zero them (wo rows there are zero,
                # but fp8 garbage could be NaN and NaN*0 poisons the psum)
                nc.gpsimd.memset(onrm[64:128, 1, :], 0.0)
                for qc in range(SQ):
                    def u(b=b, h=h, expT=expT, onrm=onrm, qc=qc):
                        vv = VV[b, h]
                        qs = slice(qc * 512, qc * 512 + 512)
                        ps0 = psAV.tile([128, 512], F32, tag="av0")
                        ps1 = psAV.tile([65, 512], F32, tag="av1")
                        for kc in range(0, ST, 2):
                            nc.tensor.matmul(
                                ps0[:],
                                lhsT=vv[:, kc : kc + 2, 0:128],
                                rhs=expT[:, kc : kc + 2, qs],
                                start=(kc == 0),
                                stop=(kc == ST - 2),
                                perf_mode=DRMODE,
                            )
                            nc.tensor.matmul(
                                ps1[:],
                                lhsT=vv[:, kc : kc + 2, 128 : DH + 1],
                                rhs=expT[:, kc : kc + 2, qs],
                                start=(kc == 0),
                                stop=(kc == ST - 2),
                                perf_mode=DRMODE,
                            )
                        rc = smal.tile([1, 512], F32, tag="rc")
                        nc.vector.reciprocal(rc[:], ps1[64:65, :])
                        rb = smal.tile([128, 512], F32, tag="rb")
                        nc.gpsimd.partition_broadcast(rb[:], rc[:])
                        nc.vector.tensor_mul(out=onrm[:, 0, qs], in0=ps0[:], in1=rb[:])
                        nc.vector.tensor_mul(
                            out=onrm[:64, 1, qs], in0=ps1[:64, :], in1=rb[:64, :]
                        )
                    units.append(u)
            return units

        def outproj_units(b, lnp):
            units = []
            for st in range(ST):
                def u(b=b, st=st):
                    ps = psP.tile([128, D], F32, tag="op")
                    for h in range(H):
                        nc.tensor.matmul(
                            ps[:],
                            lhsT=ON[b, h][:, 0:2, st * 128 : st * 128 + 128],
                            rhs=wo_sb[:, 2 * h : 2 * h + 2, :],
                            start=(h == 0),
                            stop=(h == H - 1),
                            perf_mode=DRMODE,
                        )
                    t = lnp.tile([128, D], F32, tag="t")
                    nc.vector.tensor_add(out=t[:], in0=ps[:], in1=xnbo[:, b, st, :])
                    stats = lnp.tile([128, 6], F32, tag="st")
                    nc.vector.bn_stats(out=stats[:], in_=t[:])
                    mv = lnp.tile([128, 2], F32, tag="mv")
                    nc.vector.bn_aggr(out=mv[:], in_=stats[:])
                    sd = lnp.tile([128, 1], F32, tag="sd")
                    nc.scalar.activation(
                        out=sd[:], in_=mv[:, 1:2], func=AF.Sqrt, bias=eps1_sb[:],
                    )
                    nc.vector.reciprocal(sd[:], sd[:])
                    xv = x1n[:, b, st, :]
                    nc.vector.tensor_scalar(
                        out=xv, in0=t[:], scalar1=mv[:, 0:1], scalar2=sd[:],
                        op0=ALU.subtract, op1=ALU.mult,
                    )
                    if not triv1:
                        nc.gpsimd.tensor_mul(out=xv, in0=xv, in1=g1_sb[:])
                        nc.gpsimd.tensor_add(out=xv, in0=xv, in1=be1_sb[:])
                    for dc in range(DC):
                        tp = psT.tile([128, 128], BF16, tag="tp")
                        nc.tensor.transpose(
                            tp[:], x1n[:, b, st, dc * 128 : dc * 128 + 128], ident[:]
                        )
                        nc.scalar.copy(
                            out=x1T[:, b, dc, 4 + st * 128 : 4 + st * 128 + 128],
                            in_=tp[:],
                        )
                units.append(u)
            return units

        def conv1_units(b, qcs, w1p, psF, hT, state=None):
            # `state` may arrive pre-seeded with ft=0's w1 tile (prefetched
            # during P2's tail so P3's first matmuls don't stall on DMA)
            units = []
            skip_first = state is not None and "w1" in state
            if state is None:
                state = {}
            for ft in range(FT):
                if not (ft == 0 and skip_first):
                    def udma(ft=ft, state=state):
                        w1 = w1p.tile([128, K, DC, 128], BF16, tag="w1")
                        state["w1"] = w1
                        nc.sync.dma_start(w1[:], wc1_d[:, :, :, ft * 128 : ft * 128 + 128])
                    units.append(udma)
                for qc in qcs:
                    def u(b=b, ft=ft, qc=qc, state=state):
                        w1 = state["w1"]
                        ps = psF.tile([128, 512], F32, tag="c1")
                        idx = 0
                        for k9 in range(K):
                            for dc in range(DC):
                                nc.tensor.matmul(
                                    ps[:],
                                    lhsT=w1[:, k9, dc, :],
                                    rhs=x1T[:, b, dc, qc * 512 + k9 : qc * 512 + k9 + 512],
                                    start=(idx == 0),
                                    stop=(idx == K * DC - 1),
                                )
                                idx += 1
                        nc.scalar.activation(
                            out=hT[:, ft, 4 + qc * 512 : 4 + qc * 512 + 512],
                            in_=ps[:],
                            func=AF.Relu,
                            bias=bc1_sb[:, ft : ft + 1],
                            scale=1.0,
                        )
                    units.append(u)
            return units

        def conv2(b, psG, ln2, hT, w2, x1nc7=None):
            for st in range(ST):
                ps = psG.tile([128, D], F32, tag="c2")
                idx = 0
                for k9 in range(K):
                    for fc in range(FT):
                        nc.tensor.matmul(
                            ps[:],
                            lhsT=hT[:, fc, st * 128 + k9 : st * 128 + k9 + 128],
                            rhs=w2[:, k9, fc, :],
                            start=(idx == 0),
                            stop=(idx == K * FT - 1),
                        )
                        idx += 1
                t = ln2.tile([128, D], F32, tag="t")
                if x1nc7 is not None and st == ST - 1:
                    nc.vector.tensor_add(out=t[:], in0=ps[:], in1=x1nc7[:])
                else:
                    nc.vector.tensor_add(out=t[:], in0=ps[:], in1=x1n[:, b, st, :])
                    nc.vector.tensor_add(out=t[:], in0=t[:], in1=bc2_sb[:])
                stats = ln2.tile([128, 6], F32, tag="st")
                nc.vector.bn_stats(out=stats[:], in_=t[:])
                mv = ln2.tile([128, 2], F32, tag="mv")
                nc.vector.bn_aggr(out=mv[:], in_=stats[:])
                sd = ln2.tile([128, 1], F32, tag="sd")
                nc.scalar.activation(
                    out=sd[:], in_=mv[:, 1:2], func=AF.Sqrt, bias=eps_sb[:],
                )
                nc.vector.reciprocal(sd[:], sd[:])
                ot = ln2.tile([128, D], F32, tag="o")
                nc.vector.tensor_scalar(
                    out=ot[:], in0=t[:], scalar1=mv[:, 0:1], scalar2=sd[:],
                    op0=ALU.subtract, op1=ALU.mult,
                )
                if not triv2:
                    nc.vector.tensor_mul(out=ot[:], in0=ot[:], in1=g2_sb[:])
                    nc.vector.tensor_add(out=ot[:], in0=ot[:], in1=be2_sb[:])
                nc.sync.dma_start(y_d[b, st], ot[:])

        # ================== emission ==================
        qkp0 = actx.enter_context(tc.tile_pool(name="qkp0", bufs=1))
        qkp1 = actx.enter_context(tc.tile_pool(name="qkp1", bufs=1))
        expp = actx.enter_context(tc.tile_pool(name="expp", bufs=1))
        mskp = actx.enter_context(tc.tile_pool(name="mskp", bufs=6))
        smal = actx.enter_context(tc.tile_pool(name="smal", bufs=2))
        lnp = actx.enter_context(tc.tile_pool(name="lnp", bufs=3))
        attn0 = actx.enter_context(tc.tile_pool(name="attn0", bufs=1))
        attn1 = actx.enter_context(tc.tile_pool(name="attn1", bufs=1))

        # ---- P0: qkv(b0), double-buffered through psF (conv1 not started) ----
        for u in qkv_units(0, qkp0, psF):
            u()
        # residual prefold (vector work while PE chews on attention)
        for b in range(NB):
            for st in range(ST):
                nc.vector.tensor_add(
                    out=xnbo[:, b, st, :], in0=xnbo[:, b, st, :], in1=bo_sb[:]
                )

        hT0 = hT0p.tile([128, FT, SP], BF16, tag="hT0")
        nc.gpsimd.memset(hT0[:, :, 0:4], 0.0)
        nc.gpsimd.memset(hT0[:, :, 4 + S : SP], 0.0)

        # ---- P1 ----
        A = scoreav_units(0, expp, mskp, smal, attn0)
        Bq = qkv_units(1, qkp1, psF)
        Bmain, Brsv = Bq[:-6], Bq[-6:]
        weave(A, Bmain, frac=0.85)
        # drain AV(b0) vector chains behind the qkv leftovers before outproj
        for uu in Brsv:
            uu()
        C = outproj_units(0, lnp)
        D1 = conv1_units(0, (0,), w1p, psF, hT0)
        for i in range(5):
            C[i]()
        weave(C[5:], D1, frac=0.3)

        # ---- P2 ----
        E = scoreav_units(1, expp, mskp, smal, attn1)
        Fc = conv1_units(0, (1,), w1p, psF, hT0)
        Fmain, Frsv = Fc[:-6], Fc[-6:]
        weave(E, Fmain, frac=0.8)
        G = outproj_units(1, lnp)
        weave(G, Frsv)
        # prefetch P3's first w1 tile while outproj(b1) still feeds the PE
        p3w1 = {}
        w1pf = w1p.tile([128, K, DC, 128], BF16, tag="w1")
        p3w1["w1"] = w1pf
        nc.sync.dma_start(w1pf[:], wc1_d[:, :, :, 0:128])

        # ---- P3: attention pools released; w2 takes their SBUF ----
        actx.close()
        with ExitStack() as p3:
            w2p = p3.enter_context(tc.tile_pool(name="w2p", bufs=1))
            w2 = w2p.tile([128, K, FT, D], BF16, tag="w2")
            nc.sync.dma_start(w2[:], wc2_d[:])
            hT1p = p3.enter_context(tc.tile_pool(name="hT1p", bufs=1))
            hT1 = hT1p.tile([128, FT, SP], BF16, tag="hT1")
            nc.gpsimd.memset(hT1[:, :, 0:4], 0.0)
            nc.gpsimd.memset(hT1[:, :, 4 + S : SP], 0.0)
            # prefold the very last LN2 residual to shorten the kernel tail
            x1nc7 = persist.tile([128, D], F32, tag="x1nc7")
            nc.vector.tensor_add(
                out=x1nc7[:], in0=x1n[:, 1, ST - 1, :], in1=bc2_sb[:]
            )
            for u in conv1_units(1, (0, 1), w1p, psF, hT1, state=p3w1):
                u()
            psG = p3.enter_context(tc.tile_pool(name="psG", bufs=3, space="PSUM"))
            ln2 = p3.enter_context(tc.tile_pool(name="ln2", bufs=2))
            conv2(0, psG, ln2, hT0, w2)
            conv2(1, psG, ln2, hT1, w2, x1nc7=x1nc7)


def _build(trivial_g1, trivial_g2):
    key = ("nc", trivial_g1, trivial_g2)
    if key not in _CACHE:
        nc = bacc.Bacc()
        _CACHE[key] = _emit(nc, trivial_g1, trivial_g2)
    return _CACHE[key]


def _prep_shared(Wq, bq, Wk, bk, Wv, bv, Wo, bo, Wc1, bc1, Wc2, bc2, g1, beta1, g2, beta2):
    bf = ml_dtypes.bfloat16
    f8 = ml_dtypes.float8_e4m3
    f32 = np.float32
    sh = {}
    pm = lambda w: np.ascontiguousarray(w.transpose(2, 0, 1, 3).astype(bf))
    pm8 = lambda w, s: np.ascontiguousarray(
        (w.transpose(2, 0, 1, 3) * s).astype(f8)
    )
    sh["wq"] = pm8(Wq.reshape(H, DC, 128, DH), SW)
    sh["wk"] = pm8(Wk.reshape(H, DC, 128, DH), SW)
    sh["wv"] = pm8(Wv.reshape(H, DC, 128, DH), SW)
    wo = np.zeros((4, 128, D), dtype=f8)
    bounds = ((0, 128), (128, 192), (192, 320), (320, 384))
    for c, (r0, r1) in enumerate(bounds):
        wo[c, : r1 - r0] = (Wo[r0:r1] * SWO).astype(f8)
    sh["wo"] = np.ascontiguousarray(wo.transpose(1, 0, 2))
    sh["wc1"] = pm(Wc1.reshape(K, DC, 128, F))
    sh["wc2"] = pm(Wc2.reshape(K, FT, 128, D))
    bqk = np.zeros((2, H, 2, 128), dtype=f32)
    for i, bb in enumerate((bq, bk)):
        for h in range(H):
            bqk[i, h, 0, :] = bb[h, :128] * SQK
            bqk[i, h, 1, :64] = bb[h, 128:] * SQK
    sh["bqk"] = np.ascontiguousarray(bqk.transpose(3, 0, 1, 2))
    # softmax(A)@(V+bv) == softmax(A)@V + bv, so bv folds into bo through Wo
    sh["bo"] = (CRES * (bo + bv.reshape(-1) @ Wo)).astype(f32)
    sh["bc1t"] = np.ascontiguousarray(bc1.reshape(FT, 128).T.astype(f32))
    sh["bc2"] = bc2.astype(f32)
    sh["g1"] = g1.astype(f32)
    sh["be1"] = beta1.astype(f32)
    sh["g2"] = g2.astype(f32)
    sh["be2"] = beta2.astype(f32)
    return sh


def run_sharded(inputs, trace=False):
    x = np.asarray(inputs["x"], dtype=np.float32)
    mask = np.asarray(inputs["mask"])
    sh = _prep_shared(
        *[np.asarray(inputs[k]) for k in (
            "Wq", "bq", "Wk", "bk", "Wv", "bv", "Wo", "bo",
            "Wc1", "bc1", "Wc2", "bc2", "g1", "beta1", "g2", "beta2",
        )]
    )
    triv1 = bool(
        np.all(sh["g1"] == 1.0) and np.all(sh["be1"] == 0.0)
    )
    triv2 = bool(
        np.all(sh["g2"] == 1.0) and np.all(sh["be2"] == 0.0)
    )
    nc = _build(triv1, triv2)
    bf = ml_dtypes.bfloat16
    f8 = ml_dtypes.float8_e4m3
    in_maps = []
    for c in range(NCORES):
        xb = x[c * NB : (c + 1) * NB]  # [NB, S, D]
        m = {}
        m["xT"] = (
            np.ascontiguousarray(xb.transpose(0, 2, 1)).reshape(NB, DC, 128, S)
            * SXT
        ).astype(f8)
        m["xn"] = (np.ascontiguousarray(xb.reshape(NB, ST, 128, D)) * CRES).astype(bf)
        mb = mask[c * NB : (c + 1) * NB]
        m["mT"] = np.ascontiguousarray(
            (~mb.transpose(0, 2, 1)).astype(bf)
        ).reshape(NB, ST, 128, S)
        m.update(sh)
        in_maps.append(m)
    res = run_bass_kernel_spmd(nc, in_maps, core_ids=list(range(NCORES)), trace=trace)
    out = np.empty((B, S, D), dtype=np.float32)
    for c in range(NCORES):
        out[c * NB : (c + 1) * NB] = res.results[c]["y"].reshape(NB, S, D)
    return out, res


def kernel(**inputs):
    out, _ = run_sharded(inputs, trace=False)
    return out

